# revision 37
# baseline (speedup 1.0000x reference)
"""DiT block kernel for Trainium2, data-parallel over batch (8 cores, B=8).

Layout strategy: activations are kept feature-major on chip ([H, S]; H on
partitions) so every matmul consumes them directly (contraction dim on
partitions for both operands). The host transposes x per batch element and
transposes the output back. LayerNorm statistics are computed with
ones-vector matmuls on the tensor engine (partition-axis reduction).
Per-token row vectors (rstd, mean*rstd, softmax 1/den) are transposed into
a [128, 8] token-on-partition layout through a DRAM scratch buffer (an
ExternalOutput tensor — Internal DRAM buffers fail to load under the axon
PJRT path), processed cheaply there, and broadcast back across partitions
with stride-0-partition DMA reads.

Attention: scores are computed transposed ([k, q]) per head so the AV
matmul needs no transposes; softmax denominators come for free from a ones
column appended to V in the AV matmul's stationary operand.

ACT usage is restricted to two table sets (exp_and_others for everything
up to the MLP, gelu_apprx_tanh_and_others for the MLP) so only one
activation-table reload happens per kernel.
"""

import os
import sys
import functools
from contextlib import ExitStack

import numpy as np

for _p in ("/opt/trn_rl_repo", "/root/.axon_site/_ro/trn_rl_repo"):
    if os.path.isdir(_p) and _p not in sys.path:
        sys.path.insert(0, _p)

import ml_dtypes  # noqa: E402
import concourse.bass as bass  # noqa: E402
from concourse import bacc  # noqa: E402
import concourse.tile as tile  # noqa: E402
from concourse import mybir  # noqa: E402
from concourse.bass_utils import run_bass_kernel_spmd  # noqa: E402

F32 = mybir.dt.float32
BF16 = mybir.dt.bfloat16
AF = mybir.ActivationFunctionType
OP = mybir.AluOpType

B, S, H, NH, CH = 8, 1024, 1024, 16, 64
P = 128
KH = H // P          # 8 chunks over H
KS = S // P          # 8 chunks over S
NQ = S // 512        # 2 free-dim chunks of 512
EPS = 1e-6
N_CORES = 8

# DRAM scratch layout (f32 elements) inside the "scr" ExternalOutput
SCR_CMOD = 0                      # 6*H
SCR_LN = 6 * H                    # 2 regions x 4096: sum, sq, r, mr
SCR_HEAD = SCR_LN + 2 * 4096      # per head 2048: den, rd
SCR_N = SCR_HEAD + NH * 2048
# bf16 scratch: 2 LN regions x (r 1024 | mr 1024), then per-head rd 1024
SCR2_LN = 0
SCR2_HEAD = 4096
SCR2_N = SCR2_HEAD + NH * 1024


def _build_program():
    nc = bacc.Bacc("TRN2", target_bir_lowering=False, debug=False)

    t = {}
    t["xT"] = nc.dram_tensor("xT", (H, S), F32, kind="ExternalInput").ap()
    t["cvec"] = nc.dram_tensor("cvec", (H,), F32, kind="ExternalInput").ap()
    t["w_ada"] = nc.dram_tensor("w_ada", (H, 6 * H), BF16, kind="ExternalInput").ap()
    t["b_ada"] = nc.dram_tensor("b_ada", (6 * H,), F32, kind="ExternalInput").ap()
    t["w_qkv"] = nc.dram_tensor("w_qkv", (H, 3 * H), BF16, kind="ExternalInput").ap()
    t["w_proj"] = nc.dram_tensor("w_proj", (H, H), BF16, kind="ExternalInput").ap()
    t["b_proj"] = nc.dram_tensor("b_proj", (H,), F32, kind="ExternalInput").ap()
    t["w_mlp1"] = nc.dram_tensor("w_mlp1", (H, 4 * H), BF16, kind="ExternalInput").ap()
    t["b_mlp1"] = nc.dram_tensor("b_mlp1", (4 * H,), F32, kind="ExternalInput").ap()
    # host-retiled: w_mlp2t[mc, p, ko, m] = w_mlp2[ko*128+p, mc*128+m]
    t["w_mlp2t"] = nc.dram_tensor(
        "w_mlp2t", (KH, P, 32, P), BF16, kind="ExternalInput"
    ).ap()
    t["b_mlp2"] = nc.dram_tensor("b_mlp2", (H,), F32, kind="ExternalInput").ap()
    t["outT"] = nc.dram_tensor("outT", (H, S), F32, kind="ExternalOutput").ap()
    t["scr"] = nc.dram_tensor("scr", (SCR_N,), F32, kind="ExternalOutput").ap()
    t["scr2"] = nc.dram_tensor("scr2", (SCR2_N,), BF16,
                               kind="ExternalOutput").ap()

    nrep = int(os.environ.get("KREPEAT", "1"))
    with tile.TileContext(nc) as tc:
        for _rep in range(nrep):
            _emit(tc, t, _rep)
    nc.compile()
    return nc


def _emit(tc, t, rep=0):
    nc = tc.nc
    scr = t["scr"]
    scr2 = t["scr2"]

    def pbcast(ap_1p, nparts):
        """Partition-broadcast view of a 1-partition (DRAM) AP."""
        return bass.AP(
            tensor=ap_1p.tensor, offset=ap_1p.offset,
            ap=[[0, nparts]] + list(ap_1p.ap[1:]),
        )

    def scr_row(off, n):
        """scr[off:off+n] as a [1, n] AP."""
        return scr[off:off + n].rearrange("(a n) -> a n", a=1)

    def scr_tok(off, n):
        """scr[off:off+n] as a [128, n//128] token-on-partition AP."""
        return scr[off:off + n].rearrange("(k p) -> p k", p=P)

    def scr2_row(off, n):
        return scr2[off:off + n].rearrange("(a n) -> a n", a=1)

    def scr2_tok(off, n):
        return scr2[off:off + n].rearrange("(k p) -> p k", p=P)

    with ExitStack() as ctx:
        const = ctx.enter_context(tc.tile_pool(name="const", bufs=1))
        rows = ctx.enter_context(tc.tile_pool(name="rows", bufs=1))
        work2 = ctx.enter_context(tc.tile_pool(name="work2", bufs=2))  # tmp_f32 + tmp_bf
        work4 = ctx.enter_context(tc.tile_pool(name="work4", bufs=3))
        bcast = ctx.enter_context(tc.tile_pool(name="bcast", bufs=1))
        xbpool = ctx.enter_context(tc.tile_pool(name="xbpool", bufs=8))
        psum = ctx.enter_context(tc.tile_pool(name="psum", bufs=4, space="PSUM"))

        KPH = int(os.environ.get("KPHASE", "7"))

        def dump_and_done(chunks):
            for mc in range(KH):
                dt_ = work2.tile([P, S], F32, tag="tmp_f32")
                nc.vector.tensor_copy(dt_, chunks[mc])
                nc.sync.dma_start(t["outT"][mc * P:(mc + 1) * P, :], dt_)

        ones_col = const.tile([P, 1], BF16, tag="ones_col")
        nc.vector.memset(ones_col, 1.0)

        # ---- per-partition-scalar views of biases -------------------------
        b_proj_sb = const.tile([P, KH], F32, tag="b_proj_sb")
        nc.gpsimd.dma_start(b_proj_sb, t["b_proj"].rearrange("(k p) -> p k", p=P))
        b_mlp1_sb = const.tile([P, 32], F32, tag="b_mlp1_sb")
        nc.gpsimd.dma_start(b_mlp1_sb, t["b_mlp1"].rearrange("(k p) -> p k", p=P))
        b_mlp2_sb = const.tile([P, KH], F32, tag="b_mlp2_sb")
        nc.gpsimd.dma_start(b_mlp2_sb, t["b_mlp2"].rearrange("(k p) -> p k", p=P))

        if KPH == 0:
            x0_pool0 = ctx.enter_context(tc.tile_pool(name="x00", bufs=1))
            xr0 = []
            for kc in range(KH):
                xt = x0_pool0.tile([P, S], F32, tag=f"x00_{kc}")
                nc.sync.dma_start(xt, t["xT"][kc * P:(kc + 1) * P, :])
                xr0.append(xt)
            return dump_and_done(xr0)

        # ================= adaLN: cmod = silu(c) @ w_ada + b_ada ==========
        # silu(c) = c / (1 + exp(-c)) — exp table + exact DVE reciprocal
        c_sb = const.tile([P, KH], F32, tag="c_sb")
        nc.gpsimd.dma_start(c_sb, t["cvec"].rearrange("(k p) -> p k", p=P))
        e_sb = const.tile([P, KH], F32, tag="e_sb")
        nc.scalar.activation(e_sb, c_sb, AF.Exp, scale=-1.0)
        nc.vector.tensor_scalar(out=e_sb, in0=e_sb, scalar1=1.0, scalar2=0.0,
                                op0=OP.add, op1=OP.bypass)
        nc.vector.reciprocal(e_sb, e_sb)
        sc_sb = const.tile([P, KH], BF16, tag="sc_sb")
        nc.vector.tensor_tensor(sc_sb, c_sb, e_sb, OP.mult)  # silu(c)

        cmod_a = const.tile([P, 16], F32, tag="cmod_a")
        cmod_b = const.tile([P, 8], F32, tag="cmod_b")
        cmod_c = const.tile([P, 24], F32, tag="cmod_c")
        with tc.tile_pool(name="wada", bufs=3) as wada_pool:
            brow = wada_pool.tile([1, 6 * H], F32, tag="bada_row")
            nc.gpsimd.dma_start(brow, t["b_ada"].rearrange("(a n) -> a n", a=1))
            crow = wada_pool.tile([1, 6 * H], F32, tag="cmod_row")
            for nb in range(12):
                sl = slice(nb * 512, (nb + 1) * 512)
                ps = psum.tile([P, 1024], F32, tag="ps")
                for kc in range(KH):
                    wt = wada_pool.tile([P, 512], BF16, tag="wada")
                    nc.sync.dma_start(wt, t["w_ada"][kc * P:(kc + 1) * P, sl])
                    nc.tensor.matmul(
                        ps[0:1, 0:512], lhsT=sc_sb[:, kc:kc + 1], rhs=wt,
                        start=(kc == 0), stop=(kc == KH - 1),
                    )
                nc.vector.tensor_tensor(
                    crow[0:1, sl], ps[0:1, 0:512], brow[0:1, sl], OP.add
                )
                if nb == 3:    # shift_msa | scale_msa ready
                    nc.gpsimd.dma_start(scr_row(SCR_CMOD, 2048),
                                        crow[0:1, 0:2048])
                    nc.gpsimd.dma_start(
                        cmod_a, scr[SCR_CMOD:SCR_CMOD + 2048]
                        .rearrange("(k p) -> p k", p=P))
                elif nb == 5:  # gate_msa ready
                    nc.gpsimd.dma_start(scr_row(SCR_CMOD + 2048, 1024),
                                        crow[0:1, 2048:3072])
                    nc.gpsimd.dma_start(
                        cmod_b, scr[SCR_CMOD + 2048:SCR_CMOD + 3072]
                        .rearrange("(k p) -> p k", p=P))
                elif nb == 11:  # mlp shift/scale/gate ready
                    nc.gpsimd.dma_start(scr_row(SCR_CMOD + 3072, 3072),
                                        crow[0:1, 3072:6144])
                    nc.gpsimd.dma_start(
                        cmod_c, scr[SCR_CMOD + 3072:SCR_CMOD + 6144]
                        .rearrange("(k p) -> p k", p=P))

        # cmod_a: shift_msa 0:8 | scale_msa 8:16 ; cmod_b: gate_msa 0:8
        # cmod_c: shift_mlp 0:8 | scale_mlp 8:16 | gate_mlp 16:24
        sc1 = const.tile([P, 16], F32, tag="sc1")  # 1+scale_msa | 1+scale_mlp
        nc.scalar.add(sc1[:, 0:8], cmod_a[:, 8:16], 1.0)
        nc.scalar.add(sc1[:, 8:16], cmod_c[:, 8:16], 1.0)

        # ================= load residual x (feature-major) =================
        x0_pool = ctx.enter_context(tc.tile_pool(name="x0", bufs=1))
        xres = []
        for kc in range(KH):
            xt = x0_pool.tile([P, S], F32, tag=f"x0_{kc}")
            nc.sync.dma_start(xt, t["xT"][kc * P:(kc + 1) * P, :])
            xres.append(xt)

        if KPH == 1:
            return dump_and_done(xres)

        def ln_rows(x_chunks, lnbase, name):
            """Returns (r_b, mr_b): [128,S] f32 broadcast tiles holding rstd
            and mean*rstd per token."""
            ps_sum = psum.tile([P, 1024], F32, tag="ps")
            ps_sq = psum.tile([P, 1024], F32, tag="ps")
            xbl = []
            for kc in range(KH):
                xb = xbpool.tile([P, S], BF16, tag="xb")
                nc.vector.tensor_copy(xb, x_chunks[kc])
                xbl.append(xb)
                xsq = work4.tile([P, S], BF16, tag="ln_b16")
                nc.vector.tensor_tensor(xsq, xb, xb, OP.mult)
                for q in range(NQ):
                    sl = slice(q * 512, (q + 1) * 512)
                    nc.tensor.matmul(
                        ps_sum[0:1, sl], lhsT=ones_col, rhs=xb[:, sl],
                        start=(kc == 0), stop=(kc == KH - 1),
                    )
                    nc.tensor.matmul(
                        ps_sq[0:1, sl], lhsT=ones_col, rhs=xsq[:, sl],
                        start=(kc == 0), stop=(kc == KH - 1),
                    )
            srow = rows.tile([1, S], F32, tag="srow")
            nc.scalar.copy(srow, ps_sum[0:1, :])
            qrow = rows.tile([1, S], F32, tag="qrow")
            nc.vector.tensor_copy(qrow, ps_sq[0:1, :])
            o_sum, o_sq, o_r, o_mr = (lnbase, lnbase + 1024,
                                      lnbase + 2048, lnbase + 3072)
            nc.gpsimd.dma_start(scr_row(o_sum, S), srow)
            nc.gpsimd.dma_start(scr_row(o_sq, S), qrow)
            # token-on-partition math: [128, 8]
            tsum = rows.tile([P, KS], F32, tag="tsum")
            nc.gpsimd.dma_start(tsum, scr_tok(o_sum, S))
            tsq = rows.tile([P, KS], F32, tag="tsq")
            nc.gpsimd.dma_start(tsq, scr_tok(o_sq, S))
            m = rows.tile([P, KS], F32, tag="m_tok")
            nc.vector.tensor_scalar(out=m, in0=tsum, scalar1=1.0 / H,
                                    scalar2=0.0, op0=OP.mult, op1=OP.bypass)
            v = rows.tile([P, KS], F32, tag="v_tok")
            nc.vector.tensor_scalar(out=v, in0=tsq, scalar1=1.0 / H,
                                    scalar2=0.0, op0=OP.mult, op1=OP.bypass)
            msq = rows.tile([P, KS], F32, tag="msq_tok")
            nc.vector.tensor_tensor(msq, m, m, OP.mult)
            nc.vector.tensor_tensor(v, v, msq, OP.subtract)
            nc.vector.tensor_scalar(out=v, in0=v, scalar1=EPS, scalar2=0.0,
                                    op0=OP.add, op1=OP.bypass)
            # rsqrt: linear seed r0 = 1.5 - 0.5 v (<=5% err for v in
            # [0.7, 1.4]) + 2 Newton iterations -> ~2e-5 relative
            r = rows.tile([P, KS], F32, tag="r_tok")
            nc.vector.tensor_scalar(out=r, in0=v, scalar1=-0.5, scalar2=1.5,
                                    op0=OP.mult, op1=OP.add)
            s = rows.tile([P, KS], F32, tag="s_tok")
            for _ in range(2):
                nc.vector.tensor_tensor(s, r, r, OP.mult)
                nc.vector.tensor_tensor(s, s, v, OP.mult)
                nc.vector.tensor_scalar(out=s, in0=s, scalar1=-0.5, scalar2=1.5,
                                        op0=OP.mult, op1=OP.add)
                nc.vector.tensor_tensor(r, r, s, OP.mult)
            nc.vector.tensor_tensor(m, m, r, OP.mult)  # m <- m * r
            rb16 = rows.tile([P, KS], BF16, tag="rb16")
            nc.vector.tensor_copy(rb16, r)
            mb16 = rows.tile([P, KS], BF16, tag="mb16")
            nc.vector.tensor_copy(mb16, m)
            o2_r = SCR2_LN + (0 if lnbase == SCR_LN else 2048)
            o2_mr = o2_r + 1024
            nc.gpsimd.dma_start(scr2_tok(o2_r, S), rb16)
            nc.gpsimd.dma_start(scr2_tok(o2_mr, S), mb16)
            r_b = bcast.tile([P, S], BF16, tag="r_b")
            mr_b = bcast.tile([P, S], BF16, tag="mr_b")
            nc.gpsimd.dma_start(r_b, pbcast(scr2_row(o2_r, S), P))
            nc.gpsimd.dma_start(mr_b, pbcast(scr2_row(o2_mr, S), P))
            return r_b, mr_b, xbl

        def modulate(x_chunks, r_b, mr_b, sc1_base, shift_tile, zpool, name):
            """z[kc] = ((x - m) * r) * (1 + scale) + shift, in bf16."""
            z = []
            for kc in range(KH):
                tm = work2.tile([P, S], BF16, tag="tmp_bf")
                nc.vector.tensor_tensor(tm, x_chunks[kc], r_b, OP.mult)
                nc.vector.tensor_tensor(tm, tm, mr_b, OP.subtract)
                zt = zpool.tile([P, S], BF16, tag=f"z_{name}_{kc}",
                                name=f"z_{name}_{kc}")
                nc.vector.tensor_scalar(
                    out=zt, in0=tm,
                    scalar1=sc1[:, sc1_base + kc:sc1_base + kc + 1],
                    scalar2=shift_tile[:, kc:kc + 1],
                    op0=OP.mult, op1=OP.add,
                )
                z.append(zt)
            return z

        # ================= LN1 + modulate ==================================
        r1_b, mr1_b, xb1 = ln_rows(xres, SCR_LN, "ln1")

        with tc.tile_pool(name="att_out", bufs=1) as att_out:
            kqT = [att_out.tile([P, S], BF16, tag=f"kqT_{mc}", name=f"kqT_{mc}")
                   for mc in range(16)]
            v_sb = [att_out.tile([P, NH, CH + 1], BF16, tag=f"v_{sc}",
                                 name=f"v_{sc}")
                    for sc in range(KS)]
            y_sb = [att_out.tile([P, S], BF16, tag=f"y_{mc}", name=f"y_{mc}")
                    for mc in range(KH)]
            for sc in range(KS):
                nc.vector.memset(v_sb[sc][:, :, CH:CH + 1], 1.0)

            att_ctx = ExitStack()
            wstream = att_ctx.enter_context(
                tc.tile_pool(name="wstream", bufs=2))

            def stream_w(fn):
                ws = []
                for kc in range(KH):
                    wt = wstream.tile([P, H], BF16, tag=f"w3_{kc}",
                                      name=f"w3_{kc}")
                    nc.sync.dma_start(wt, fn(kc))
                    ws.append(wt)
                return ws

            with tc.tile_pool(name="z1_pool", bufs=1) as z1_pool:
                z1 = modulate(xb1, r1_b, mr1_b, 0, cmod_a, z1_pool, "msa")
                if KPH == 2:
                    return dump_and_done(z1)

                # ---- k^T and q^T (feature-major [H, S]) -------------------
                for third in range(2):
                    if True:
                        w_sb = stream_w(lambda kc, th=third: t["w_qkv"][
                            kc * P:(kc + 1) * P, th * H:(th + 1) * H])
                        for m in range(KH):
                            mc = third * KH + m
                            ps = psum.tile([P, 1024], F32, tag="ps")
                            for kc in range(KH):
                                for q in range(NQ):
                                    sl = slice(q * 512, (q + 1) * 512)
                                    nc.tensor.matmul(
                                        ps[:, sl],
                                        lhsT=w_sb[kc][:, m * P:(m + 1) * P],
                                        rhs=z1[kc][:, sl],
                                        start=(kc == 0), stop=(kc == KH - 1),
                                    )
                            if mc % 2 == 0:
                                nc.vector.tensor_copy(kqT[mc], ps)
                            else:
                                nc.scalar.copy(kqT[mc], ps)

                # ---- v (token-major [S, H] + ones column per head) --------
                if True:
                    wv_sb = stream_w(
                        lambda kc: t["w_qkv"][kc * P:(kc + 1) * P, 2 * H:3 * H])
                    for sc in range(KS):
                        ps = psum.tile([P, 1024], F32, tag="ps")
                        for kc in range(KH):
                            for q in range(NQ):
                                sl = slice(q * 512, (q + 1) * 512)
                                nc.tensor.matmul(
                                    ps[:, sl],
                                    lhsT=z1[kc][:, sc * P:(sc + 1) * P],
                                    rhs=wv_sb[kc][:, sl],
                                    start=(kc == 0), stop=(kc == KH - 1),
                                )
                        nc.vector.tensor_copy(
                            v_sb[sc][:, :, 0:CH],
                            ps.rearrange("p (h c) -> p h c", h=NH),
                        )

            if KPH == 3:
                return dump_and_done(kqT[0:8])

            # ================= attention, head by head =====================
            with tc.tile_pool(name="att_tmp", bufs=2) as att_tmp, \
                 tc.tile_pool(name="wexp_pool", bufs=8) as wexp_pool:
                for h in range(NH):
                    mk = h // 2
                    off = (h % 2) * CH
                    o_den = SCR_HEAD + h * 2048
                    o_rd = o_den + 1024
                    # scores^T in [k, q] layout; exp fused with 1/CH scale
                    wexp = []
                    for kc in range(KS):
                        ps_s = psum.tile([P, 1024], F32, tag="ps")
                        for q in range(NQ):
                            sl = slice(q * 512, (q + 1) * 512)
                            nc.tensor.matmul(
                                ps_s[:, sl],
                                lhsT=kqT[mk][off:off + CH, kc * P:(kc + 1) * P],
                                rhs=kqT[8 + mk][off:off + CH, sl],
                                start=True, stop=True,
                            )
                        we = wexp_pool.tile([P, S], BF16, tag="wexp")
                        if kc % 2 == 0:
                            nc.scalar.activation(we, ps_s, AF.Exp, scale=1.0 / CH)
                        else:
                            sb = work2.tile([P, S], BF16, tag="tmp_bf")
                            nc.vector.tensor_scalar(
                                out=sb, in0=ps_s, scalar1=1.0 / CH, scalar2=0.0,
                                op0=OP.mult, op1=OP.bypass)
                            nc.scalar.activation(we, sb, AF.Exp)
                        wexp.append(we)
                    # AV with ones column: rows 0:CH = y_unnorm^T, row CH = den
                    ps_y = psum.tile([P, 1024], F32, tag="ps")
                    for kc in range(KS):
                        for q in range(NQ):
                            sl = slice(q * 512, (q + 1) * 512)
                            nc.tensor.matmul(
                                ps_y[0:CH + 1, sl],
                                lhsT=v_sb[kc][:, h, :],
                                rhs=wexp[kc][:, sl],
                                start=(kc == 0), stop=(kc == KS - 1),
                            )
                    drow = att_tmp.tile([1, S], F32, tag="drow")
                    nc.vector.tensor_copy(drow, ps_y[CH:CH + 1, :])
                    yun = att_tmp.tile([CH, S], BF16, tag="yun")
                    nc.scalar.copy(yun, ps_y[0:CH, :])
                    nc.gpsimd.dma_start(scr_row(o_den, S), drow)
                    dtok = att_tmp.tile([P, KS], F32, tag="dtok")
                    nc.gpsimd.dma_start(dtok, scr_tok(o_den, S))
                    rtok = att_tmp.tile([P, KS], F32, tag="rtok")
                    nc.vector.reciprocal(rtok, dtok)
                    rt16 = att_tmp.tile([P, KS], BF16, tag="rt16")
                    nc.vector.tensor_copy(rt16, rtok)
                    o2_rd = SCR2_HEAD + h * 1024
                    nc.gpsimd.dma_start(scr2_tok(o2_rd, S), rt16)
                    rdb = att_tmp.tile([CH, S], BF16, tag="rdb")
                    nc.sync.dma_start(rdb, pbcast(scr2_row(o2_rd, S), CH))
                    nc.vector.tensor_tensor(
                        y_sb[mk][off:off + CH, :], yun, rdb, OP.mult
                    )

            if KPH == 4:
                return dump_and_done(y_sb)

            # ================= proj + gated residual (in place) ============
            if True:
                wproj_sb = stream_w(
                    lambda kc: t["w_proj"][kc * P:(kc + 1) * P, :])
                for mc in range(KH):
                    ps = psum.tile([P, 1024], F32, tag="ps")
                    for kc in range(KH):
                        for q in range(NQ):
                            sl = slice(q * 512, (q + 1) * 512)
                            nc.tensor.matmul(
                                ps[:, sl],
                                lhsT=wproj_sb[kc][:, mc * P:(mc + 1) * P],
                                rhs=y_sb[kc][:, sl],
                                start=(kc == 0), stop=(kc == KH - 1),
                            )
                    tp = work2.tile([P, S], F32, tag="tmp_f32")
                    nc.vector.tensor_scalar(
                        out=tp, in0=ps,
                        scalar1=b_proj_sb[:, mc:mc + 1],
                        scalar2=cmod_b[:, mc:mc + 1],
                        op0=OP.add, op1=OP.mult,
                    )
                    nc.vector.tensor_tensor(xres[mc], xres[mc], tp, OP.add)
            att_ctx.close()

        if KPH == 5:
            return dump_and_done(xres)

        # ================= LN2 + modulate + MLP ============================
        r2_b, mr2_b, xb2 = ln_rows(xres, SCR_LN + 4096, "ln2")

        with tc.tile_pool(name="h_pool", bufs=1) as h_pool:
            h_sb = [h_pool.tile([P, S], BF16, tag=f"h_{mc}", name=f"h_{mc}")
                    for mc in range(32)]

            mlp2_ctx = ExitStack()
            wm2_pool = mlp2_ctx.enter_context(tc.tile_pool(name="wm2", bufs=2))
            mlp_ctx = ExitStack()
            wm1s = mlp_ctx.enter_context(tc.tile_pool(name="wm1s", bufs=2))
            with tc.tile_pool(name="z2_pool", bufs=1) as z2_pool:
                z2 = modulate(xb2, r2_b, mr2_b, 8, cmod_c, z2_pool, "mlp")
                for grp in range(4):
                    w_sb = []
                    for kc in range(KH):
                        wt = wm1s.tile([P, H], BF16, tag=f"wm1_{kc}",
                                       name=f"wm1_{kc}")
                        nc.sync.dma_start(
                            wt,
                            t["w_mlp1"][kc * P:(kc + 1) * P,
                                        grp * H:(grp + 1) * H],
                        )
                        w_sb.append(wt)
                    for m in range(KH):
                        mc = grp * KH + m
                        ps = psum.tile([P, 1024], F32, tag="ps")
                        for kc in range(KH):
                            for q in range(NQ):
                                sl = slice(q * 512, (q + 1) * 512)
                                nc.tensor.matmul(
                                    ps[:, sl],
                                    lhsT=w_sb[kc][:, m * P:(m + 1) * P],
                                    rhs=z2[kc][:, sl],
                                    start=(kc == 0), stop=(kc == KH - 1),
                                )
                        nc.scalar.activation(
                            h_sb[mc], ps, AF.Gelu_apprx_tanh,
                            bias=b_mlp1_sb[:, mc:mc + 1],
                        )
            mlp_ctx.close()

            if KPH == 6:
                return dump_and_done(h_sb[0:KH])

            with tc.tile_pool(name="out_pool", bufs=3) as out_pool:
                for mc in range(KH):
                    wt = wm2_pool.tile([P, 32, P], BF16, tag="wm2")
                    nc.sync.dma_start(wt, t["w_mlp2t"][mc])
                    ps = psum.tile([P, 1024], F32, tag="ps")
                    for kc in range(32):
                        for q in range(NQ):
                            sl = slice(q * 512, (q + 1) * 512)
                            nc.tensor.matmul(
                                ps[:, sl],
                                lhsT=wt[:, kc, :],
                                rhs=h_sb[kc][:, sl],
                                start=(kc == 0), stop=(kc == 31),
                            )
                    tm = work2.tile([P, S], F32, tag="tmp_f32")
                    nc.vector.tensor_scalar(
                        out=tm, in0=ps,
                        scalar1=b_mlp2_sb[:, mc:mc + 1],
                        scalar2=cmod_c[:, 16 + mc:16 + mc + 1],
                        op0=OP.add, op1=OP.mult,
                    )
                    ot = out_pool.tile([P, S], F32, tag="out_t")
                    nc.vector.tensor_tensor(ot, xres[mc], tm, OP.add)
                    nc.sync.dma_start(t["outT"][mc * P:(mc + 1) * P, :], ot)
            mlp2_ctx.close()


@functools.lru_cache(maxsize=1)
def _get_nc():
    return _build_program()


def kernel(x, c, w_ada, b_ada, w_qkv, w_proj, b_proj, w_mlp1, b_mlp1,
           w_mlp2, b_mlp2):
    nc = _get_nc()
    bf = ml_dtypes.bfloat16
    w_mlp2t = np.ascontiguousarray(
        np.asarray(w_mlp2, dtype=bf)
        .reshape(32, P, KH, P)     # [ko, p, mc, m]
        .transpose(2, 1, 0, 3)     # -> [mc, p, ko, m]
    )
    shared = {
        "w_ada": np.ascontiguousarray(w_ada, dtype=bf),
        "b_ada": np.ascontiguousarray(b_ada, dtype=np.float32),
        "w_qkv": np.ascontiguousarray(w_qkv, dtype=bf),
        "w_proj": np.ascontiguousarray(w_proj, dtype=bf),
        "b_proj": np.ascontiguousarray(b_proj, dtype=np.float32),
        "w_mlp1": np.ascontiguousarray(w_mlp1, dtype=bf),
        "b_mlp1": np.ascontiguousarray(b_mlp1, dtype=np.float32),
        "w_mlp2t": w_mlp2t,
        "b_mlp2": np.ascontiguousarray(b_mlp2, dtype=np.float32),
    }
    in_maps = []
    for bidx in range(N_CORES):
        m = dict(shared)
        m["xT"] = np.ascontiguousarray(np.asarray(x[bidx], dtype=np.float32).T)
        m["cvec"] = np.ascontiguousarray(np.asarray(c[bidx], dtype=np.float32))
        in_maps.append(m)

    trace = bool(int(os.environ.get("KERNEL_TRACE", "0")))
    res = run_bass_kernel_spmd(
        nc, in_maps, core_ids=list(range(N_CORES)), trace=trace
    )
    kernel.last_results = res

    out = np.empty((B, S, H), dtype=np.float32)
    for bidx in range(N_CORES):
        out[bidx] = np.asarray(res.results[bidx]["outT"]).T
    return out


if __name__ == "__main__":
    nc = _get_nc()
    print("program built ok")
